# revision 1
# baseline (speedup 1.0000x reference)
"""MinGRU Trainium2 kernel.

Reference computation (B=8, T=4096, D=H=1024):
    k        = x @ W_z.T + b_z
    z        = sigmoid(k);  coeff = 1 - z
    tilde_h  = g(x @ W_h.T + b_h)   where g(u) = max(u + 0.5, sigmoid(u))
    h_t      = coeff_t * h_{t-1} + z_t * tilde_h_t,  h_init = g(h_0)
    output   = [g(h_0), h_1 .. h_T]  per batch  -> [B, T+1, H]

The reference evaluates the scan in log space purely for numerical
stability; the direct-space recurrence is a convex combination at every
step (contraction), so fp32 direct evaluation matches to ~1e-6.

Sharding: data-parallel over batch. Core b computes batch b; there is no
cross-core communication.

Precision strategy (validated on hardware, rel err 1.50e-2 vs the 2e-2
gate, matching the host simulation):
  - z-path matmul (x @ W_z.T) runs in fp8 e4m3 with perf_mode=DoubleRow:
    each instruction contracts two 128-row k-blocks (2x MAC throughput),
    so the K=1024 contraction is 4 instructions instead of 8. W_z is
    pre-scaled by 32 on the host so its entries (|w|<1/32) land in fp8's
    normal range; the 1/32 is folded into the sigmoid's activation scale.
    The sigmoid saturates, so fp8 error on this path stays bounded.
  - h-path matmul (x @ W_h.T) runs in float32r (full rate, ~1.6e-4
    relative): it feeds tilde_h directly so it needs the precision.
  - coeff = 1 - z is computed as a second sigmoid with negated
    scale/bias (exact identity), keeping it on the Scalar engine.
  - time recurrence as hardware TensorTensorScan in fp32.

Schedule notes (from perfetto traces):
  - PSUM is split into dedicated 4-bank pools (z-path / h-path) so a
    bank's reuse is gated by readers a full chunk earlier, never by the
    congested Vector queue of the current chunk.
  - x prefetches are per-k-block DMAs (one big 3D DMA serializes on a
    single DGE ring) issued at the TOP of the previous chunk, ahead of
    that chunk's scan-gated output DMAs on the Sync queue.
  - The Vector queue is software-pipelined: scan(h-1) is issued after
    stt(h), so the Vector engine runs stt(h) while GpSimd produces
    v(h-1), instead of idle-waiting scan(h-1) -> stt(h) in FIFO order.
  - The last chunk interleaves the two matmul sweeps per h-block so the
    scan drain overlaps the matmul stream instead of trailing it.
"""

import numpy as np

B, T, D, H = 8, 4096, 1024, 1024
NCORES = 8
PB = 128          # partition block
KB = D // PB      # contraction blocks (8)
KP = KB // 2      # DoubleRow contraction pair-blocks (4)
HB = H // PB      # output-row blocks (8)
TCHUNK = 512      # moving free-dim per matmul / scan chunk
NT = T // TCHUNK  # 8 time chunks
WSCALE = 32.0     # host pre-scale on W_z before fp8 quantization

_cache = {}


def _build_bass():
    import concourse.tile as tile
    import concourse.mybir as mybir
    from concourse import bacc

    f32 = mybir.dt.float32
    f32r = mybir.dt.float32r
    f8 = mybir.dt.float8e4
    ACT = mybir.ActivationFunctionType
    OP = mybir.AluOpType
    DR = mybir.MatmulPerfMode.DoubleRow

    nc = bacc.Bacc("TRN2", target_bir_lowering=False, debug=False,
                   num_devices=NCORES)

    x8T = nc.dram_tensor("x8T", [D, T], f8, kind="ExternalInput")
    xT = nc.dram_tensor("xT", [D, T], f32r, kind="ExternalInput")
    wz8T = nc.dram_tensor("wz8T", [D, H], f8, kind="ExternalInput")
    whT = nc.dram_tensor("whT", [D, H], f32r, kind="ExternalInput")
    # packed per-partition constants, one column per 128-row H block:
    # [b_z | b_h+0.5 | b_h | g(h0) | -b_z]
    consts = nc.dram_tensor("consts", [PB, 5 * HB], f32, kind="ExternalInput")
    hT = nc.dram_tensor("hT", [H, T], f32, kind="ExternalOutput")

    with tile.TileContext(nc) as tc:
        with (
            tc.tile_pool(name="wpool", bufs=1) as wpool,
            tc.tile_pool(name="cpool", bufs=1) as cpool,
            tc.tile_pool(name="xpool", bufs=2) as xpool,
            tc.tile_pool(name="zpool", bufs=10) as zpool,
            tc.tile_pool(name="spool", bufs=6) as spool,
            tc.tile_pool(name="hpool", bufs=2) as hpool,
            tc.tile_pool(name="zpsum", bufs=4, space="PSUM") as zpsum,
            tc.tile_pool(name="hpsum", bufs=4, space="PSUM") as hpsum,
        ):
            cb = cpool.tile([PB, 5 * HB], f32, tag="consts")
            nc.sync.dma_start(cb[:], consts[:])

            def bias_bz(h):
                return cb[:, h:h + 1]

            def bias_bh05(h):
                return cb[:, HB + h:HB + h + 1]

            def bias_bh(h):
                return cb[:, 2 * HB + h:2 * HB + h + 1]

            def init_g0(h):
                return cb[:, 3 * HB + h:3 * HB + h + 1]

            def bias_nbz(h):
                return cb[:, 4 * HB + h:4 * HB + h + 1]

            # chunk-0 DMAs in consumption order: fp8 z-path data first so
            # the first DoubleRow matmul gates on ~0.4 MB, then per-k
            # (xf, wh) pairs so the h-path sweep starts k-by-k
            x8_tiles = [None] * NT
            xf_tiles = [None] * NT
            x8_tiles[0] = xpool.tile([PB, KB, TCHUNK], f8, tag="x8", name="x8_0")
            wz_sb = wpool.tile([PB, KB, H], f8, tag="wz")
            for k in range(KB):
                nc.sync.dma_start(
                    x8_tiles[0][:, k, :], x8T[k * PB:(k + 1) * PB, 0:TCHUNK])
                nc.sync.dma_start(wz_sb[:, k, :], wz8T[k * PB:(k + 1) * PB, :])
            # chunk-0 f32r data is issued on the (still idle) Scalar and
            # GpSimd queues so it doesn't serialize behind the z-path
            # issues on Sync; a one-column read of the last fp8 k-block
            # gates these 6 MB of transfers until the z-sweep's 1.5 MB has
            # landed, so they don't steal its HBM bandwidth
            wh_sb = wpool.tile([PB, KB, H], f32r, tag="wh")
            xf_tiles[0] = xpool.tile([PB, KB, TCHUNK], f32r, tag="xf",
                                     name="xf_0")
            gate1 = cpool.tile([PB, 1], f32, tag="gate1")
            gate2 = cpool.tile([PB, 1], f32, tag="gate2")
            nc.scalar.copy(gate1[:], x8_tiles[0][:, KB - 1, 0:1])
            nc.gpsimd.tensor_copy(gate2[:], x8_tiles[0][:, KB - 1, 0:1])
            for k in range(KB):
                nc.scalar.dma_start(
                    xf_tiles[0][:, k, :], xT[k * PB:(k + 1) * PB, 0:TCHUNK])
                nc.gpsimd.dma_start(wh_sb[:, k, :], whT[k * PB:(k + 1) * PB, :])

            def prefetch_x8(t):
                ns0 = t * TCHUNK
                x8_tiles[t] = xpool.tile([PB, KB, TCHUNK], f8,
                                         tag="x8", name=f"x8_{t}")
                for k in range(KB):
                    nc.sync.dma_start(
                        x8_tiles[t][:, k, :],
                        x8T[k * PB:(k + 1) * PB, ns0:ns0 + TCHUNK])

            def prefetch_xf(t):
                ns0 = t * TCHUNK
                xf_tiles[t] = xpool.tile([PB, KB, TCHUNK], f32r,
                                         tag="xf", name=f"xf_{t}")
                for k in range(KB):
                    nc.sync.dma_start(
                        xf_tiles[t][:, k, :],
                        xT[k * PB:(k + 1) * PB, ns0:ns0 + TCHUNK])

            prefetch_x8(1)
            prefetch_xf(1)

            def mm_z(pt, h, x8_sb, kp):
                hs = slice(h * PB, (h + 1) * PB)
                nc.tensor.matmul(pt[:], wz_sb[:, 2 * kp:2 * kp + 2, hs],
                                 x8_sb[:, 2 * kp:2 * kp + 2, :],
                                 start=(kp == 0), stop=(kp == KP - 1),
                                 perf_mode=DR)

            def mm_h(pt, h, xf_sb, k):
                hs = slice(h * PB, (h + 1) * PB)
                nc.tensor.matmul(pt[:], wh_sb[:, k, hs], xf_sb[:, k, :],
                                 start=(k == 0), stop=(k == KB - 1))

            def z_and_c(pk, h):
                z = zpool.tile([PB, TCHUNK], f32, tag="z")
                nc.scalar.activation(z[:], pk[:], ACT.Sigmoid,
                                     bias=bias_bz(h),
                                     scale=float(1.0 / WSCALE))
                c = zpool.tile([PB, TCHUNK], f32, tag="c")
                nc.scalar.activation(c[:], pk[:], ACT.Sigmoid,
                                     bias=bias_nbz(h),
                                     scale=float(-1.0 / WSCALE))
                return z, c

            def tilde_and_v(pp, h, z):
                sp = spool.tile([PB, TCHUNK], f32, tag="sp")
                nc.scalar.activation(sp[:], pp[:], ACT.Sigmoid,
                                     bias=bias_bh(h), scale=1.0)
                # tilde = max(pre + b_h + 0.5, sigmoid(pre + b_h))
                tilde = spool.tile([PB, TCHUNK], f32, tag="tilde")
                nc.vector.scalar_tensor_tensor(
                    tilde[:], pp[:], bias_bh05(h), sp[:],
                    op0=OP.add, op1=OP.max)
                v = spool.tile([PB, TCHUNK], f32, tag="v")
                nc.gpsimd.tensor_mul(v[:], z[:], tilde[:])
                return v

            h_prev = [None] * HB

            def scan_and_store(t, h, c, v):
                hout = hpool.tile([PB, TCHUNK], f32, tag=f"h{h}",
                                  name=f"h_{t}_{h}")
                init = (init_g0(h) if t == 0
                        else h_prev[h][:, TCHUNK - 1:TCHUNK])
                nc.vector.tensor_tensor_scan(
                    hout[:], c[:], v[:], init,
                    op0=OP.mult, op1=OP.add)
                h_prev[h] = hout
                hs = slice(h * PB, (h + 1) * PB)
                nc.sync.dma_start(hT[hs, t * TCHUNK:(t + 1) * TCHUNK], hout[:])

            for t in range(NT - 1):
                x8_sb = x8_tiles[t]
                xf_sb = xf_tiles[t]
                k_outer = (t == 0)

                # ---- W_z sweep (fp8 DoubleRow): z and coeff ----
                zs, cs = [None] * HB, [None] * HB
                pks = [None] * HB
                for kp, h in (((k_, h_) for k_ in range(KP) for h_ in range(HB))
                              if k_outer else
                              ((k_, h_) for h_ in range(HB) for k_ in range(KP))):
                    if pks[h] is None:
                        pks[h] = zpsum.tile([PB, TCHUNK], f32, tag="zps",
                                            name=f"pk_{t}_{h}")
                    mm_z(pks[h], h, x8_sb, kp)
                    if kp == KP - 1:
                        zs[h], cs[h] = z_and_c(pks[h], h)

                # ---- W_h sweep (f32r): tilde, v, then pipelined scans ----
                vs = [None] * HB
                pps = [None] * HB
                done = []  # h-blocks whose v is issued, scan not yet
                for k, h in (((k_, h_) for k_ in range(KB) for h_ in range(HB))
                             if k_outer else
                             ((k_, h_) for h_ in range(HB) for k_ in range(KB))):
                    if pps[h] is None:
                        pps[h] = hpsum.tile([PB, TCHUNK], f32, tag="hps",
                                            name=f"pp_{t}_{h}")
                    mm_h(pps[h], h, xf_sb, k)
                    if k != KB - 1:
                        continue
                    vs[h] = tilde_and_v(pps[h], h, zs[h])
                    done.append(h)
                    # scan lags two blocks behind stt: scan(h) enters the
                    # Vector FIFO after stt(h+2), so GpSimd's v(h+1) runs
                    # during scan(h) instead of waiting behind it
                    if len(done) >= 3:
                        hp = done.pop(0)
                        scan_and_store(t, hp, cs[hp], vs[hp])
                # prefetch chunk t+2's x between this chunk's in-loop
                # output DMAs and the drain's scan-gated ones, so the
                # fp8 z-path data is never stuck behind a scan wait on
                # the Sync queue (xf follows after the drain; its WAR on
                # chunk t's in-flight reads resolves by then)
                if t + 2 < NT:
                    prefetch_x8(t + 2)
                for hp in done:
                    scan_and_store(t, hp, cs[hp], vs[hp])
                if t + 2 < NT:
                    prefetch_xf(t + 2)

            # ---- last chunk: interleave the sweeps per h-block (z-path
            # matmuls first so z/c activations overlap the h-path matmuls)
            # and keep the whole side chain on the Vector FIFO (v included,
            # no scan lag) -- in-order on one engine, the drain after the
            # final matmul is just one block's pipeline depth ----
            t = NT - 1
            x8_sb = x8_tiles[t]
            xf_sb = xf_tiles[t]
            for h in range(HB):
                pk = zpsum.tile([PB, TCHUNK], f32, tag="zps", name=f"pk_{t}_{h}")
                for kp in range(KP):
                    mm_z(pk, h, x8_sb, kp)
                pp = hpsum.tile([PB, TCHUNK], f32, tag="hps", name=f"pp_{t}_{h}")
                for k in range(KB):
                    mm_h(pp, h, xf_sb, k)
                z, c = z_and_c(pk, h)
                sp = spool.tile([PB, TCHUNK], f32, tag="sp")
                nc.scalar.activation(sp[:], pp[:], ACT.Sigmoid,
                                     bias=bias_bh(h), scale=1.0)
                tilde = spool.tile([PB, TCHUNK], f32, tag="tilde")
                nc.vector.scalar_tensor_tensor(
                    tilde[:], pp[:], bias_bh05(h), sp[:],
                    op0=OP.add, op1=OP.max)
                v = spool.tile([PB, TCHUNK], f32, tag="v")
                nc.vector.tensor_mul(v[:], z[:], tilde[:])
                scan_and_store(t, h, c, v)

    nc.compile()
    return nc


def _get_nc():
    if "nc" not in _cache:
        _cache["nc"] = _build_bass()
    return _cache["nc"]


def _prep_inputs(x, h_0, W_z, b_z, W_h, b_h):
    import ml_dtypes

    f8 = ml_dtypes.float8_e4m3

    x = np.asarray(x, dtype=np.float32)
    h_0 = np.asarray(h_0, dtype=np.float32)
    W_z = np.asarray(W_z, dtype=np.float32)
    b_z = np.asarray(b_z, dtype=np.float32)
    W_h = np.asarray(W_h, dtype=np.float32)
    b_h = np.asarray(b_h, dtype=np.float32)

    wz8T = np.ascontiguousarray((W_z.T * np.float32(WSCALE)).astype(f8))
    whT = np.ascontiguousarray(W_h.T)

    h0f = h_0.reshape(B, H)
    g0 = np.where(h0f >= 0.0, h0f + np.float32(0.5),
                  1.0 / (1.0 + np.exp(-h0f))).astype(np.float32)  # [B, H]

    def blocked(vec):  # [H] -> [PB, HB] column per block
        return np.ascontiguousarray(vec.reshape(HB, PB).T)

    in_maps = []
    for b in range(B):
        consts = np.concatenate(
            [blocked(b_z), blocked(b_h + np.float32(0.5)), blocked(b_h),
             blocked(g0[b]), blocked(-b_z)], axis=1).astype(np.float32)
        xT = np.ascontiguousarray(x[b].T)
        in_maps.append({
            "x8T": np.ascontiguousarray(xT.astype(f8)),   # [D, T]
            "xT": xT,                                     # [D, T]
            "wz8T": wz8T, "whT": whT,
            "consts": consts,
        })
    return in_maps, g0


def kernel(x, h_0, W_z, b_z, W_h, b_h):
    import time
    from concourse.bass_utils import run_bass_kernel_spmd

    in_maps, g0 = _prep_inputs(x, h_0, W_z, b_z, W_h, b_h)
    nc = _get_nc()
    out = np.empty((B, T + 1, H), dtype=np.float32)
    for attempt in range(4):
        try:
            res = run_bass_kernel_spmd(nc, in_maps, core_ids=list(range(NCORES)))
        except Exception:
            # transient NRT device errors (e.g. NRT_EXEC_UNIT_UNRECOVERABLE)
            # recover on retry once the runtime resets the core
            if attempt == 3:
                raise
            time.sleep(5)
            continue
        _cache["last_results"] = res
        for b in range(B):
            out[b, 0, :] = g0[b]
            out[b, 1:, :] = res.results[b]["hT"].T
        # guard against rare startup races: h is a convex combination of
        # values in (0, ~4), so NaN or large magnitudes mean a poisoned
        # run -- rerun instead of returning garbage
        if np.isnan(out).any() or np.abs(out).max() > 50.0:
            if attempt == 3:
                break
            continue
        break
    return out

